# revision 16
# baseline (speedup 1.0000x reference)
"""AttnTopKPool Trainium2 kernel.

reference:
    w_mean = mean(w, axis=1)          # [B, S, S] -> [B, S]
    idx    = top_k(w_mean, 16)        # [B, 16]
    out    = x[b, :, idx[b]]          # [B, F, 16]

Strategy (8 NeuronCores, batch-parallel, 4 batches each):
  - host: transpose x to x_t[b, s, f] so the device gather is a contiguous
    row gather; slice w and x_t per core.
  - device: stream all 64 MiB of w per core on the sync HWDGE queue as
    sixteen 4 MiB [128, 8192] loads. HW-probed facts driving this:
      * descriptor->SDMA-engine assignment is a blocked split of the
        partition dim over n_engines = largest divisor of n_partitions
        <= 16; only the 128-partition shape (8 partition-lines per
        engine) hits full per-engine rate (~26.5 GB/s); 120/124/126-
        partition loads drop to 13-21 GB/s per engine.
      * the machine phase-shifts run to run: SDMA engine 15 sometimes
        runs ~19% slower (21.6 vs 26.5 GB/s; it always carries 1/16 of
        [128,*] loads, flooring the stream at ~194 us), and sometimes
        the DVE clock throttles ~20%. Fast-phase stream is ~160 us.
        With wt bufs=5 the buffer for each load is freed ~2 loads early
        and the straggler engine stays saturated the whole stream.
  - reduction at QUARTER granularity: each pair of loads is summed by
    DVE in [128, 2048] quarter-adds (~2.3 us) into paq tiles, each
    immediately consumed by 4 fp32 [128,512] matmuls against a ones
    vector accumulating into the batch's [1, 2048] PSUM row (bank =
    s-block; 8 accumulations per bank per batch). fp32 is required: the
    smallest top-16 gap on these U(0,1) column sums is ~4e-3 while
    fp32r/TF32 matmul error is ~5e-3 (HW-probed).
  - top-k ops are emitted interleaved with the NEXT batch's quarter-adds
    so the Tile list scheduler never convoys the adds (which free stream
    buffers) behind top-k ops waiting on the PE.
  - top-16 ordering: max8 -> match_replace -> max8 -> find_index8(1st 8)
    -> find_index8(2nd 8). find_index of pass 1 commutes past
    match_replace/max8 of pass 2 (match_replace only reads PSUM), which
    cuts the critical path to the LAST gathers from 11.5 us to 9.2 us.
  - tail: the last batch's slot 14 self-pairs (fr0+fr2, fr1+fr3) and
    slot 15 streams as four 1 MiB sub-loads into one wt tile, self-paired
    (q0+q1, q2+q3), so after the final byte only one 2.3 us add + 8
    small matmuls precede top-k.
  - gather: per index, reg_load into a register and issue a
    dynamic-offset DMA copying that 4 KiB row of x_t[b] straight
    DRAM->DRAM into the output, on scalar+gpsimd (never the streaming
    sync queue; +sync for the last batch once the stream is done).
  - out per core: [64, 1024] = (b_loc*16 + k, f); host reassembles to
    [B, F, K].
"""

import numpy as np

B, F, S, K = 32, 1024, 2048, 16
N_CORES = 8
B_LOC = B // N_CORES  # 4
P = 128
ROWS_PER_PART = 4          # w rows per SBUF partition in one big load
LOAD_FREE = ROWS_PER_PART * S   # 8192 floats = 32 KiB per partition
QF = S                     # 2048: quarter-add free size
MM_N = 512                 # one PSUM bank of fp32
NQ = QF // MM_N            # 4 psum column slices per quarter
NEG = -3.0e38              # below any column sum

_cached_nc = None

# test-only knobs (harness leaves these at defaults)
TRACE = False
_last_results = None


def _build_nc():
    from concourse import bacc, bass, mybir, tile

    f32 = mybir.dt.float32
    u32 = mybir.dt.uint32

    nc = bacc.Bacc("TRN2", target_bir_lowering=False, debug=False)

    w_d = nc.dram_tensor("w", [B_LOC, S, S], f32, kind="ExternalInput")
    xt_d = nc.dram_tensor("xt", [B_LOC, S, F], f32, kind="ExternalInput")
    out_d = nc.dram_tensor("out", [B_LOC * K, F], f32, kind="ExternalOutput")

    w_rows = w_d[:].rearrange("b r s -> (b r) s")
    # big view: [16, 128, 8192]; partition p of slot t holds rows (512t + 4p ..+3)
    w_big = w_rows.rearrange("(t p fr) s -> t p (fr s)", p=P, fr=ROWS_PER_PART)
    LAST = B_LOC - 1

    with tile.TileContext(nc) as tc:
        with (
            tc.tile_pool(name="wtp", bufs=5) as wtp,
            tc.tile_pool(name="paq", bufs=4) as paqp,
            tc.tile_pool(name="smpool", bufs=1) as smpool,
            tc.tile_pool(name="pspool", bufs=2, space="PSUM") as pspool,
            tc.tile_pool(name="tk", bufs=1) as tk,
        ):
            ones = tk.tile([P, 1], f32)
            nc.vector.memset(ones[:], 1.0)

            ps_of = {}
            paq_ctr = [0]

            def quarter_add(b, qidx, a_ap, b_ap):
                """DVE add of two [128, QF] slices into a paq tile, then 4
                matmul chunks into psum banks 0..3. qidx 0..7 per batch."""
                ps = ps_of[b]
                pa = paqp.tile([P, QF], f32, name=f"pa{paq_ctr[0]}", tag="paq")
                paq_ctr[0] += 1
                nc.vector.tensor_add(pa[:], a_ap, b_ap)
                for c in range(NQ):
                    nc.tensor.matmul(
                        ps[:, c * MM_N : (c + 1) * MM_N],
                        ones[:],
                        pa[:, c * MM_N : (c + 1) * MM_N],
                        start=(qidx == 0),
                        stop=(qidx == 7),
                    )

            def gather(b, k, etype, eng, gidx):
                regs = nc.alloc_registers(name=f"ri{b}_{k}", engines=(etype,))
                reg = list(regs)[0]
                eng.reg_load(reg, gidx[0:1, k : k + 1])
                val = eng.snap(reg, donate=True, min_val=0, max_val=S - 1)
                eng.dma_start(
                    out_d[b * K + k : b * K + k + 1, :],
                    xt_d[b][bass.ds(val, 1), :],
                )

            def make_topk(b):
                """Three stages: s1 = max8+match_replace+max8 (6.9 us DVE),
                s2 = find_index8 of ranks 1-8 + their gathers, s3 =
                find_index8 of ranks 9-16 + their gathers."""
                ps = ps_of[b]
                gidx = tk.tile([1, K], u32, name=f"gidx{b}")
                m8a = tk.tile([1, 8], f32, name=f"m8a{b}")
                m8b = tk.tile([1, 8], f32, name=f"m8b{b}")
                sums = smpool.tile([1, S], f32, name=f"sums{b}", tag="sums")
                engs = [
                    (mybir.EngineType.Activation, nc.scalar),
                    (mybir.EngineType.Pool, nc.gpsimd),
                ]
                if b == LAST:
                    engs.append((mybir.EngineType.SP, nc.sync))

                def s1():
                    nc.vector.max(m8a[:], ps[:])
                    nc.vector.match_replace(sums[:], m8a[:], ps[:], NEG)
                    nc.vector.max(m8b[:], sums[:])

                def s2():
                    nc.vector.max_index(gidx[:, 0:8], m8a[:], ps[:])
                    for k in range(8):
                        gather(b, k, *engs[k % len(engs)], gidx)

                def s3():
                    nc.vector.max_index(gidx[:, 8:16], m8b[:], sums[:])
                    for k in range(8, K):
                        gather(b, k, *engs[k % len(engs)], gidx)

                return s1, s2, s3

            prev = None  # pending top-k stages of the previous batch
            for b in range(B_LOC):
                ps_of[b] = pspool.tile([1, S], f32, name=f"ps{b}", tag="ps")

                # --- first pair: two 4 MiB loads, 4 quarter-adds ---
                wa = wtp.tile([P, LOAD_FREE], f32, name=f"wa{b}", tag="wt")
                nc.sync.dma_start(wa[:], w_big[4 * b])
                wb = wtp.tile([P, LOAD_FREE], f32, name=f"wb{b}", tag="wt")
                nc.sync.dma_start(wb[:], w_big[4 * b + 1])
                for q in range(4):
                    quarter_add(
                        b, q, wa[:, q * QF : (q + 1) * QF], wb[:, q * QF : (q + 1) * QF]
                    )

                if prev is not None:
                    prev[0]()  # previous batch's max8/match_replace/max8

                wc = wtp.tile([P, LOAD_FREE], f32, name=f"wc{b}", tag="wt")
                nc.sync.dma_start(wc[:], w_big[4 * b + 2])
                wd = wtp.tile([P, LOAD_FREE], f32, name=f"wd{b}", tag="wt")
                if b != LAST:
                    # --- second pair, as the first ---
                    nc.sync.dma_start(wd[:], w_big[4 * b + 3])
                    for q in range(4):
                        quarter_add(
                            b,
                            4 + q,
                            wc[:, q * QF : (q + 1) * QF],
                            wd[:, q * QF : (q + 1) * QF],
                        )
                    if prev is not None:
                        prev[1]()
                        prev[2]()
                else:
                    # tail: wc self-pairs (fr0+fr2, fr1+fr3); wd streams as
                    # four 1 MiB sub-loads into disjoint quarters, self-
                    # paired (q0+q1, q2+q3) so only one add + 8 matmuls
                    # follow the final byte.
                    for q in range(4):
                        nc.sync.dma_start(
                            wd[:, q * QF : (q + 1) * QF],
                            w_big[4 * b + 3][:, q * QF : (q + 1) * QF],
                        )
                    quarter_add(b, 4, wc[:, 0:QF], wc[:, 2 * QF : 3 * QF])
                    quarter_add(b, 5, wc[:, QF : 2 * QF], wc[:, 3 * QF : 4 * QF])
                    if prev is not None:
                        prev[1]()
                    quarter_add(b, 6, wd[:, 0:QF], wd[:, QF : 2 * QF])
                    quarter_add(b, 7, wd[:, 2 * QF : 3 * QF], wd[:, 3 * QF : 4 * QF])
                    if prev is not None:
                        prev[2]()

                prev = make_topk(b)

            prev[0]()
            prev[1]()
            prev[2]()

    nc.compile()
    return nc


def _get_nc():
    global _cached_nc
    if _cached_nc is None:
        _cached_nc = _build_nc()
    return _cached_nc


def kernel(x: np.ndarray, w: np.ndarray) -> np.ndarray:
    from concourse import bass_utils

    x = np.asarray(x, dtype=np.float32)
    w = np.asarray(w, dtype=np.float32)
    x_t = np.ascontiguousarray(x.transpose(0, 2, 1))  # [B, S, F]

    nc = _get_nc()
    in_maps = [
        {
            "w": np.ascontiguousarray(w[c * B_LOC : (c + 1) * B_LOC]),
            "xt": x_t[c * B_LOC : (c + 1) * B_LOC],
        }
        for c in range(N_CORES)
    ]
    res = bass_utils.run_bass_kernel_spmd(
        nc, in_maps, list(range(N_CORES)), trace=TRACE
    )
    global _last_results
    _last_results = res
    out = np.concatenate([res.results[c]["out"] for c in range(N_CORES)], axis=0)
    # [B*K, F] -> [B, K, F] -> [B, F, K]
    return np.ascontiguousarray(out.reshape(B, K, F).transpose(0, 2, 1))
